# revision 33
# baseline (speedup 1.0000x reference)
"""Multi-head GAT layer (entmax15 attention over fixed-degree mailbox) on 8 trn2 cores.

Strategy (per core, dst-node sharded; full inputs in, full output out):
  - Each core owns N/8 destination nodes and their DEG=16 incoming edges.
  - ONE dma_gather per 128-node tile from a paired-row table in EDGE-major
    slot order (partition p = (dst%8)*16 + d, row = dst//8): row r =
    [h_r | h_{r+25088}-h_r] (512B), so int16 indices cover all 50k nodes.
  - Logit biases P_src = h.U_src and Q_dst = h.U_dst are dense per-node
    projections folded on the HOST into pq448 (like wq448), so no P table,
    no stage A, and rows carry only features.
  - The lo/hi select is folded into the FMA: since s is 0/1,
    sum((lo+s*d)*a) = lo^T@A + d^T@(s*A), so no DVE blend exists at all;
    A2 = A*s costs one small DVE multiply per tile.
  - entmax15 (dst-major, chunk-wide): Batcher sort network + segmented
    cumsum via tensor_tensor_scan + threshold, all on DVE.
  - FMA + projection run on the TENSOR engine: alpha [dst,(h,d)] is
    PE-transposed, spread along j=dst%8 by a constant replication matmul
    (Aprec[p,(h,dst)] = alphaT[(h,d(p)),dst]), block-diag-masked on DVE,
    then 32 PSUM-accumulating matmuls (lo_g^T@A_g + d_g^T@A2_g) and 4
    projection matmuls produce out[dst, 64] with no DVE reduce tree.
  - Per-chunk software pipeline: PE work of chunk sc-1 interleaves with
    blends of chunk sc; gathers stream on GPSIMD across 4 SWDGE queues.
"""

import os
import sys
import numpy as np

sys.path.insert(0, "/opt/trn_rl_repo")

import concourse.bass as bass
import concourse.bacc as bacc
import concourse.tile as tile
from concourse import mybir
from concourse.masks import make_identity
from concourse.tile_rust import add_dep_helper
import ml_dtypes

F32 = mybir.dt.float32
BF16 = mybir.dt.bfloat16
I16 = mybir.dt.int16
ALU = mybir.AluOpType


# --- patch: make Tile's DMASW lane assignment respect SWDGE queue_num.
import concourse.tile_sem_assignment as _tsa


def _patched_assign_tick(self, inst):
    import concourse.bass_isa as bass_isa_
    engine = inst.engine
    if (isinstance(inst, _tsa.DMAInst)
            and not isinstance(inst, bass_isa_.UserSyncedRemoteDMADescs)
            and engine == mybir.EngineType.Pool):
        q = getattr(inst, "queue_num", 0) or 0
        tog = getattr(self, "_gat_q_toggle", None)
        if tog is None:
            tog = self._gat_q_toggle = [0, 0, 0, 0]
        lane = q + 4 * tog[q]
        tog[q] ^= 1
        self.next_sw_dma_idx = lane
    return _tsa.TileClockTick._orig_assign_tick(self, inst)


if not hasattr(_tsa.TileClockTick, "_orig_assign_tick"):
    _tsa.TileClockTick._orig_assign_tick = _tsa.TileClockTick._assign_tick
    _tsa.TileClockTick._assign_tick = _patched_assign_tick

# ---------------------------------------------------------------- config

N = 50000
DEG = 16
DIN = 128
DOUT = 64
H = 4
CORES = 8
NROW = 25088        # paired rows: row r covers nodes r and r+NROW
NALL = 2 * NROW     # 50176 padded nodes
ROWE = 256          # bf16 elems per row (512B): h_lo 128 | h_hi-h_lo 128


class Cfg:
    def __init__(self, gdt=BF16):
        self.n_own = 6272
        self.T = 49
        self.S = 7
        self.NCH = 7
        self.gdt = BF16


def full_cfg(gdt=BF16):
    return Cfg()


# ---------------------------------------------------------------- sort network

def batcher_stages(n=16):
    stages = []
    p = 1
    while p < n:
        k = p
        while k >= 1:
            stage = []
            for j in range(k % p, n - k, 2 * k):
                for i in range(min(k, n - j - k)):
                    if (i + j) // (p * 2) == (i + j + k) // (p * 2):
                        stage.append((i + j, i + j + k))
            stages.append((k, stage))
            k //= 2
        p *= 2
    return stages


def group_lo(los):
    los = sorted(los)
    n = len(los)
    if n == 1:
        return los[0], [[1, 1], [1, 1]]
    d = [los[i + 1] - los[i] for i in range(n - 1)]
    r = 1
    while r < n and d[r - 1] == d[0]:
        r += 1
    istride = d[0]
    if r == n:
        return los[0], [[istride * n, 1], [istride, n]]
    assert n % r == 0, (los,)
    ostride = los[r] - los[0]
    for b in range(n // r):
        for i in range(r):
            assert los[b * r + i] == los[0] + b * ostride + i * istride, (los,)
    return los[0], [[ostride, n // r], [istride, r]]


SORT_STAGES = [(k, group_lo([lo for lo, _ in st])) for (k, st) in batcher_stages(16)]

# per-stage uncovered positions (copythrough in the ping-pong sort), as
# (lo, dims) AP fragments over the 16-wide neighbor axis
SORT_UNC = {
    2: (0, [[4, 4], [3, 2]]),
    4: (0, [[8, 2], [6, 2], [1, 2]]),
    5: (0, [[8, 2], [7, 2]]),
    7: (0, [[12, 2], [1, 4]]),
    8: (0, [[14, 2], [1, 2]]),
    9: (0, [[15, 2]]),
}


# ---------------------------------------------------------------- AP helper

def sub_ap(base_ap, off, dims):
    return bass.AP(
        tensor=base_ap.tensor,
        offset=base_ap.offset + off,
        ap=[list(base_ap.ap[0])] + [list(d) for d in dims],
    )


# ---------------------------------------------------------------- program

def build_program(cfg, num_devices=CORES):
    nc = bacc.Bacc("TRN2", target_bir_lowering=False, debug=False,
                   num_devices=num_devices,
                   dynamic_dma_scratch_size=int(os.environ.get("GAT_DMA_SCRATCH", 65536)),
                   num_swdge_queues=4)
    T, S, NCH = cfg.T, cfg.S, cfg.NCH
    W = S * 64
    NCK = NALL // 128   # 392 chunks for the P pass

    # ---- DRAM tensors
    h_tab = nc.dram_tensor("h_tab", [NROW, ROWE], BF16, kind="ExternalInput").ap()
    fc_wT = nc.dram_tensor("fc_wT", [H, DIN, DOUT], BF16, kind="ExternalInput").ap()
    pq_d = nc.dram_tensor("pq448", [128, T * 64], F32, kind="ExternalInput").ap()
    idx_d = nc.dram_tensor("idxP", [128, T * 128], I16, kind="ExternalInput").ap()
    selm_d = nc.dram_tensor("selm", [128, T * DEG * 8], BF16, kind="ExternalInput").ap()
    wq_d = nc.dram_tensor("wq448", [128, T * 64], F32, kind="ExternalInput").ap()
    kinv_d = nc.dram_tensor("kinv448", [128, W], F32, kind="ExternalInput").ap()
    ws_d = nc.dram_tensor("ws448", [128, W], F32, kind="ExternalInput").ap()
    smask_d = nc.dram_tensor("scanmask", [128, W], F32, kind="ExternalInput").ap()
    rh_d = nc.dram_tensor("rhrep", [64, H * 128], BF16, kind="ExternalInput").ap()
    blk_d = nc.dram_tensor("blkmask", [128, H * 128], BF16, kind="ExternalInput").ap()
    out_d = nc.dram_tensor("out", [cfg.n_own, DOUT], F32, kind="ExternalOutput").ap()

    from contextlib import ExitStack
    with tile.TileContext(nc) as tc, ExitStack() as ctx:
        singles = ctx.enter_context(tc.tile_pool(name="singles", bufs=1))

        kinv_sb = singles.tile([128, W], F32)
        ws_sb = singles.tile([128, W], F32)
        smask_sb = singles.tile([128, W], F32)
        selm_sb = singles.tile([128, T * DEG * 8], BF16)
        fcwT_sb = singles.tile([128, H * DOUT], BF16)
        ident = singles.tile([128, 128], BF16)
        rh_sb = singles.tile([64, H * 128], BF16)
        blk_sb = singles.tile([128, H * 128], BF16)
        zero_sb = singles.tile([128, W], F32)
        nc.vector.memset(zero_sb[:], 0.0)
        nc.sync.dma_start(out=rh_sb[:], in_=rh_d)
        nc.sync.dma_start(out=blk_sb[:], in_=blk_d)

        nc.sync.dma_start(out=kinv_sb[:], in_=kinv_d)
        nc.sync.dma_start(out=ws_sb[:], in_=ws_d)
        nc.sync.dma_start(out=smask_sb[:], in_=smask_d)
        nc.sync.dma_start(out=selm_sb[:], in_=selm_d)
        for hh in range(H):
            nc.sync.dma_start(out=fcwT_sb[:, hh * DOUT:(hh + 1) * DOUT], in_=fc_wT[hh])
        make_identity(nc, ident[:])

        # ---- stage B pools
        ga_pool = ctx.enter_context(tc.tile_pool(name="ga", bufs=12))
        arep_pool = ctx.enter_context(tc.tile_pool(name="arep", bufs=2))
        ck_pool = ctx.enter_context(tc.tile_pool(name="ck", bufs=2))
        sc_pool = ctx.enter_context(tc.tile_pool(name="cks", bufs=1))
        wqp = ctx.enter_context(tc.tile_pool(name="wqp", bufs=4))
        idx_pool = ctx.enter_context(tc.tile_pool(name="idx", bufs=2))
        m_pool = ctx.enter_context(tc.tile_pool(name="m", bufs=2))
        mt_pool = ctx.enter_context(tc.tile_pool(name="mt", bufs=4))
        ob_pool = ctx.enter_context(tc.tile_pool(name="ob", bufs=2))
        tr_pool = ctx.enter_context(tc.tile_pool(name="tr", bufs=1, space="PSUM"))
        apr_pool = ctx.enter_context(tc.tile_pool(name="apr", bufs=2, space="PSUM"))
        mtp_pool = ctx.enter_context(tc.tile_pool(name="mtp", bufs=2, space="PSUM"))
        asb_pool = ctx.enter_context(tc.tile_pool(name="asb", bufs=4))
        pr_pool = ctx.enter_context(tc.tile_pool(name="pr", bufs=2, space="PSUM"))

        zs_t = sc_pool.tile([128, W], F32, tag="zs")
        A_t = sc_pool.tile([128, W], F32, tag="A")
        B_t = sc_pool.tile([128, W], F32, tag="B")
        C_t = sc_pool.tile([128, W], F32, tag="C")
        ts4_t = sc_pool.tile([128, S * 4], F32, tag="ts4")

        def do_fma(prev, tl):
            """alpha spread via replication matmul + FMA and projection on PE."""
            gas_p, z_p, sc_p = prev
            t_glob = sc_p * S + tl
            ga_p = gas_p[tl]
            ab = arep_pool.tile([128, 64], BF16, tag="arep")
            nc.scalar.copy(out=ab[:], in_=z_p[:, tl * 64:(tl + 1) * 64])
            trA = tr_pool.tile([64, 128], BF16, tag="trA")
            nc.tensor.transpose(out=trA[:], in_=ab[:], identity=ident[:])
            aT = mt_pool.tile([64, 128], BF16, tag="mt")
            nc.scalar.copy(out=aT[:], in_=trA[:])
            # Aprec[p=(j,d), (h, dstl)] = alphaT[(h, d(p)), dstl]
            apr = apr_pool.tile([128, H * 128], F32, tag="apr")
            for hh in range(H):
                nc.tensor.matmul(out=apr[:, hh * 128:(hh + 1) * 128],
                                 lhsT=rh_sb[:, hh * 128:(hh + 1) * 128],
                                 rhs=aT[:], start=True, stop=True)
            # block-diag mask (zero where j(p) != dstl%8), f32->bf16,
            # permuted to (g, h, j) so each group's 32 rhs cols are contiguous
            A_sb = asb_pool.tile([128, H * 128], BF16, tag="asb")
            nc.vector.tensor_mul(
                out=sub_ap(A_sb[:], 0, [[8, H], [32, 16], [1, 8]]),
                in0=sub_ap(apr[:], 0, [[128, H], [8, 16], [1, 8]]),
                in1=sub_ap(blk_sb[:], 0, [[128, H], [8, 16], [1, 8]]))
            # A2 = A * s (selects hi-half via the delta columns)
            A2_sb = asb_pool.tile([128, H * 128], BF16, tag="asb")
            nc.vector.tensor_mul(
                out=sub_ap(A2_sb[:], 0, [[32, 16], [8, H], [1, 8]]),
                in0=sub_ap(A_sb[:], 0, [[32, 16], [8, H], [1, 8]]),
                in1=sub_ap(selm_sb[:], t_glob * DEG * 8,
                           [[8, 16], [0, H], [1, 8]]))
            # mT[f, g*32+h*8+j] = lo_g^T @ A_g + delta_g^T @ A2_g  (blend folded)
            mT = mtp_pool.tile([128, 512], F32, tag="mtp")
            for g in range(16):
                nc.tensor.matmul(
                    out=mT[:, g * 32:(g + 1) * 32],
                    lhsT=ga_p[:, g * ROWE:g * ROWE + DIN],
                    rhs=A_sb[:, g * 32:(g + 1) * 32],
                    start=True, stop=False)
                nc.tensor.matmul(
                    out=mT[:, g * 32:(g + 1) * 32],
                    lhsT=ga_p[:, g * ROWE + DIN:(g + 1) * ROWE],
                    rhs=A2_sb[:, g * 32:(g + 1) * 32],
                    start=False, stop=True)
            mTs = m_pool.tile([128, 512], BF16, tag="m")
            nc.scalar.copy(
                out=sub_ap(mTs[:], 0, [[128, H], [8, 16], [1, 8]]),
                in_=sub_ap(mT[:], 0, [[8, H], [32, 16], [1, 8]]))
            proj = pr_pool.tile([128, DOUT], F32, tag="pr")
            for hh in range(H):
                nc.tensor.matmul(out=proj[:],
                                 lhsT=mTs[:, hh * 128:(hh + 1) * 128],
                                 rhs=fcwT_sb[:, hh * DOUT:(hh + 1) * DOUT],
                                 start=(hh == 0), stop=(hh == H - 1))
            osb = ob_pool.tile([128, DOUT], F32, tag="ob")
            nc.scalar.copy(out=osb[:], in_=proj[:])
            nc.sync.dma_start(out=out_d[t_glob * 128:(t_glob + 1) * 128, :],
                              in_=osb[:])

        prev = None
        for sc in range(NCH):
            idx_sb = idx_pool.tile([128, S * 128], I16, tag="idx")
            nc.sync.dma_start(out=idx_sb[:],
                              in_=idx_d[:, sc * S * 128:(sc + 1) * S * 128])
            wq_sb = wqp.tile([128, W], F32, tag="wq")
            nc.sync.dma_start(out=wq_sb[:], in_=wq_d[:, sc * W:(sc + 1) * W])
            pq_sb = wqp.tile([128, W], F32, tag="pq")
            nc.sync.dma_start(out=pq_sb[:], in_=pq_d[:, sc * W:(sc + 1) * W])
            pt = ck_pool.tile([128, W], F32, tag="pt")

            gas = []
            for tl in range(S):
                t_glob = sc * S + tl
                ga = ga_pool.tile([128, DEG * ROWE], BF16, tag="ga")
                for gh in range(2):
                    o3 = bass.AP(tensor=ga[:].tensor,
                                 offset=ga[:].offset + gh * 8 * ROWE,
                                 ap=[list(ga[:].ap[0]), [ROWE, 8], [1, ROWE]])
                    g = nc.gpsimd.dma_gather(
                        out_ap=o3, in_ap=h_tab,
                        idxs_ap=idx_sb[:, tl * 128 + gh * 64:tl * 128 + (gh + 1) * 64],
                        num_idxs=8 * 128, num_idxs_reg=8 * 128,
                        elem_size=ROWE, single_packet=False,
                        queue_num=(2 * t_glob + gh) % 4)
                gas.append(ga)

            # ---- chunk ops: logits, sort, entmax -> alpha (in place in pt)
            z = pt
            nc.vector.scalar_tensor_tensor(out=z[:], in0=pq_sb[:], scalar=0.01,
                                           in1=pq_sb[:], op0=ALU.mult, op1=ALU.max)
            nc.vector.tensor_add(out=z[:], in0=z[:], in1=wq_sb[:])

            # sort descending: ping-pong zs_t <-> C_t, 2 DVE ops per stage
            # (+ small DVE copythrough on partially-covered stages)
            nc.scalar.copy(out=zs_t[:], in_=z[:])
            bufs_pp = [zs_t, C_t]
            for si, (k, (lo0, dims)) in enumerate(SORT_STAGES):
                src = bufs_pp[si % 2]
                dst = bufs_pp[1 - si % 2]
                ap_dims = [[16, S * 4]] + [[d[0], d[1]] for d in dims]
                a_s = sub_ap(src[:], lo0, ap_dims)
                b_s = sub_ap(src[:], lo0 + k, ap_dims)
                nc.vector.tensor_tensor(out=sub_ap(dst[:], lo0, ap_dims),
                                        in0=a_s, in1=b_s, op=ALU.max)
                nc.vector.tensor_tensor(out=sub_ap(dst[:], lo0 + k, ap_dims),
                                        in0=a_s, in1=b_s, op=ALU.min)
                if si in SORT_UNC:
                    ulo, udims = SORT_UNC[si]
                    uap = [[16, S * 4]] + [[d[0], d[1]] for d in udims]
                    u_s = sub_ap(src[:], ulo, uap)
                    nc.vector.tensor_tensor(out=sub_ap(dst[:], ulo, uap),
                                            in0=u_s, in1=u_s, op=ALU.max)
            # 10 stages (even) -> sorted result ends back in zs_t

            # segmented cumsums via scan: state = mask*state + x
            nc.vector.tensor_tensor_scan(out=A_t[:], data0=smask_sb[:], data1=zs_t[:],
                                         initial=0.0, op0=ALU.mult, op1=ALU.add)
            nc.vector.tensor_mul(out=C_t[:], in0=zs_t[:], in1=zs_t[:])
            nc.vector.tensor_tensor_scan(out=B_t[:], data0=smask_sb[:], data1=C_t[:],
                                         initial=0.0, op0=ALU.mult, op1=ALU.add)

            # entmax threshold
            nc.vector.tensor_mul(out=C_t[:], in0=A_t[:], in1=A_t[:])
            nc.vector.tensor_mul(out=C_t[:], in0=C_t[:], in1=kinv_sb[:])
            nc.vector.tensor_sub(out=C_t[:], in0=B_t[:], in1=C_t[:])      # ss
            nc.vector.tensor_mul(out=B_t[:], in0=C_t[:], in1=kinv_sb[:])
            nc.vector.tensor_sub(out=B_t[:], in0=kinv_sb[:], in1=B_t[:])  # (1-ss)/k
            nc.vector.tensor_tensor(out=B_t[:], in0=B_t[:], in1=zero_sb[:], op=ALU.max)
            nc.scalar.sqrt(out=B_t[:], in_=B_t[:])
            nc.vector.tensor_mul(out=A_t[:], in0=A_t[:], in1=kinv_sb[:])  # mean
            nc.vector.tensor_sub(out=A_t[:], in0=A_t[:], in1=B_t[:])      # tau

            nc.vector.tensor_tensor(out=C_t[:], in0=A_t[:], in1=zs_t[:], op=ALU.is_le)
            e15 = [[64, S], [16, 4], [1, DEG - 1]]
            nc.vector.tensor_sub(out=sub_ap(B_t[:], 0, e15),
                                 in0=sub_ap(C_t[:], 0, e15),
                                 in1=sub_ap(C_t[:], 1, e15))
            nc.scalar.copy(out=sub_ap(B_t[:], DEG - 1, [[64, S], [16, 4], [1, 1]]),
                           in_=sub_ap(C_t[:], DEG - 1, [[64, S], [16, 4], [1, 1]]))
            nc.vector.tensor_mul(out=B_t[:], in0=B_t[:], in1=A_t[:])
            nc.vector.tensor_reduce(
                out=sub_ap(ts4_t[:], 0, [[4, S], [1, 4]]),
                in_=sub_ap(B_t[:], 0, [[64, S], [16, 4], [1, DEG]]),
                axis=mybir.AxisListType.X, op=ALU.add)
            nc.scalar.copy(
                out=sub_ap(C_t[:], 0, [[64, S], [16, 4], [1, 16]]),
                in_=sub_ap(ts4_t[:], 0, [[4, S], [1, 4], [0, 16]]))   # tau* rep
            nc.vector.tensor_sub(out=z[:], in0=z[:], in1=C_t[:])
            nc.vector.tensor_tensor(out=z[:], in0=z[:], in1=zero_sb[:], op=ALU.max)
            nc.vector.tensor_mul(out=z[:], in0=z[:], in1=z[:])
            nc.vector.tensor_mul(out=z[:], in0=z[:], in1=ws_sb[:])

            for tl in range(S):
                if prev is not None:
                    do_fma(prev, tl)

            prev = (gas, z, sc)

        for tl in range(S):
            do_fma(prev, tl)

    nc.compile()
    return nc


# ---------------------------------------------------------------- host prep

def softmax_np(x):
    e = np.exp(x - np.max(x))
    return e / e.sum()


def host_prep(cfg, h, src, w, fc_w, attn_w, head_weights, n_cores, n_total=N):
    n_own_real = n_total // n_cores
    T, S = cfg.T, cfg.S
    W = S * 64

    h_pad = np.zeros((NALL, DIN), np.float32)
    h_pad[:n_total] = h
    hq = h_pad.astype(ml_dtypes.bfloat16)

    # paired gather table: row r = [h_r | h_{r+NROW} - h_r]
    h_tab = np.zeros((NROW, ROWE), ml_dtypes.bfloat16)
    h_tab[:, 0:DIN] = hq[:NROW]
    h_tab[:, DIN:2 * DIN] = (hq[NROW:].astype(np.float32)
                             - hq[:NROW].astype(np.float32)).astype(ml_dtypes.bfloat16)

    fc_wT = np.ascontiguousarray(np.transpose(fc_w, (0, 2, 1))).astype(ml_dtypes.bfloat16)

    # dense per-node logit projections (f32, host): P = h.U_src, Q = h.U_dst
    fc_w32 = fc_w.astype(np.float32)
    U_src = np.stack([fc_w32[hh].T @ attn_w[hh, :DOUT] for hh in range(H)], 1)
    U_dst = np.stack([fc_w32[hh].T @ attn_w[hh, DOUT:] for hh in range(H)], 1)
    P_all = h_pad @ U_src          # [NALL, H]
    Q_all = h_pad @ U_dst          # [NALL, H]

    ws = softmax_np(head_weights.astype(np.float32))
    hcol = np.arange(W) % 64                       # within-tile col = h*16+d
    h_of = hcol // DEG
    d_of = hcol % DEG
    kinv448 = np.tile((1.0 / (d_of + 1.0))[None, :], (128, 1)).astype(np.float32)
    ws448 = np.tile(ws[h_of][None, :], (128, 1)).astype(np.float32)
    smask = np.tile((d_of != 0).astype(np.float32)[None, :], (128, 1))

    src2d = src.reshape(n_total, DEG).astype(np.int64)
    w2d = w.reshape(n_total, DEG).astype(np.float32)

    i1024 = np.arange(1024)
    dh_ = i1024 // 128
    ph_ = i1024 % 128

    q64 = np.arange(64)
    p128 = np.arange(128)
    rh = np.zeros((64, H * 128), np.float32)
    for hh in range(H):
        rh[:, hh * 128:(hh + 1) * 128] = (
            (q64[:, None] // 16 == hh) & (q64[:, None] % 16 == p128[None, :] % 16))
    rhrep = rh.astype(ml_dtypes.bfloat16)
    blk = np.zeros((128, H * 128), np.float32)
    for hh in range(H):
        blk[:, hh * 128:(hh + 1) * 128] = (p128[None, :] % 8 == p128[:, None] // 16)
    blkmask = blk.astype(ml_dtypes.bfloat16)

    in_maps = []
    for c in range(n_cores):
        lo = c * n_own_real
        hi = lo + n_own_real
        own_src = np.zeros((cfg.n_own, DEG), np.int64)
        own_src[:n_own_real] = src2d[lo:hi]
        own_w = np.zeros((cfg.n_own, DEG), np.float32)
        own_w[:n_own_real] = 0.5 * w2d[lo:hi]

        sel = (own_src >= NROW)
        row = np.where(sel, own_src - NROW, own_src)

        # edge-major slots: out partition p = (dst%8)*16 + d, row v = dst//8
        idxP = np.zeros((128, T * 128), np.int16)
        for t in range(T):
            for gh in range(2):
                dstl = (8 * gh + dh_) * 8 + ph_ // 16
                vals = row[t * 128 + dstl, ph_ % 16].astype(np.int16)
                pat = np.zeros((16, 64), np.int16)
                pat[i1024 % 16, i1024 // 16] = vals
                idxP[:, t * 128 + gh * 64:t * 128 + (gh + 1) * 64] = \
                    np.tile(pat, (8, 1))

        sf = sel.astype(np.float32)                       # [n_own, DEG]
        jj = np.arange(128) // 16
        dd16 = np.arange(128) % 16
        sfr = sf.reshape(T, 16, 8, DEG)
        selE = sfr[:, :, jj, dd16].transpose(2, 0, 1)     # [128, T, 16] edge-major
        selm = np.ascontiguousarray(
            np.repeat(selE.reshape(128, T * DEG)[:, :, None], 8, axis=2)
            .reshape(128, T * DEG * 8)).astype(ml_dtypes.bfloat16)

        # pq448: [p, t*64 + h*16 + d] = 0.5*(P[src] + Q[dst])
        own_nodes = np.zeros(cfg.n_own, np.int64)
        own_nodes[:n_own_real] = np.arange(lo, hi)
        pq = 0.5 * (P_all[own_src] + Q_all[own_nodes][:, None, :])  # [n_own, DEG, H]
        pq = pq.transpose(0, 2, 1).reshape(T, 128, 64).transpose(1, 0, 2)
        pq448 = np.ascontiguousarray(pq.reshape(128, T * 64)).astype(np.float32)

        # wq448: [p, t*64 + h*16 + d] = 0.5*w[node(t,p), d]
        w3 = own_w.reshape(T, 128, DEG).transpose(1, 0, 2)   # [128, T, DEG]
        wq448 = np.tile(w3[:, :, None, :], (1, 1, H, 1)).reshape(128, T * 64)
        wq448 = np.ascontiguousarray(wq448).astype(np.float32)

        in_maps.append({
            "h_tab": h_tab, "fc_wT": fc_wT, "pq448": pq448,
            "idxP": idxP, "selm": selm, "rhrep": rhrep, "blkmask": blkmask,
            "wq448": wq448, "kinv448": kinv448, "ws448": ws448,
            "scanmask": smask,
        })
    return in_maps


# ---------------------------------------------------------------- entry point

_PROG_CACHE = {}


def kernel(h, src, w, fc_w, attn_w, head_weights):
    h = np.asarray(h, np.float32)
    src = np.asarray(src)
    w = np.asarray(w, np.float32)
    fc_w = np.asarray(fc_w, np.float32)
    attn_w = np.asarray(attn_w, np.float32)
    head_weights = np.asarray(head_weights, np.float32)

    cfg = full_cfg()
    key = ("full",)
    if key not in _PROG_CACHE:
        _PROG_CACHE[key] = build_program(cfg, num_devices=CORES)
    nc = _PROG_CACHE[key]

    in_maps = host_prep(cfg, h, src, w, fc_w, attn_w, head_weights, CORES)

    from concourse.bass_utils import run_bass_kernel_spmd
    res = run_bass_kernel_spmd(nc, in_maps, core_ids=list(range(CORES)))

    n_own_real = N // CORES
    out = np.concatenate(
        [res.results[c]["out"][:n_own_real] for c in range(CORES)], axis=0)
    return out.astype(np.float32)



# revision 34
# speedup vs baseline: 1.0701x; 1.0701x over previous
"""Multi-head GAT layer (entmax15 attention over fixed-degree mailbox) on 8 trn2 cores.

Strategy (per core, dst-node sharded; full inputs in, full output out):
  - Each core owns N/8 destination nodes and their DEG=16 incoming edges.
  - ONE dma_gather per 128-node tile from a paired-row table in EDGE-major
    slot order (partition p = (dst%8)*16 + d, row = dst//8): row r =
    [h_r | h_{r+25088}-h_r] (512B), so int16 indices cover all 50k nodes.
  - Logit biases P_src = h.U_src and Q_dst = h.U_dst are dense per-node
    projections folded on the HOST into pq448 (like wq448), so no P table,
    no stage A, and rows carry only features.
  - The lo/hi select is folded into the FMA: since s is 0/1,
    sum((lo+s*d)*a) = lo^T@A + d^T@(s*A), so no DVE blend exists at all;
    A2 = A*s costs one small DVE multiply per tile.
  - entmax15 (dst-major, chunk-wide): Batcher sort network + segmented
    cumsum via tensor_tensor_scan + threshold, all on DVE.
  - FMA + projection run on the TENSOR engine: alpha [dst,(h,d)] is
    PE-transposed, spread along j=dst%8 by a constant replication matmul
    (Aprec[p,(h,dst)] = alphaT[(h,d(p)),dst]), block-diag-masked on DVE,
    then 32 PSUM-accumulating matmuls (lo_g^T@A_g + d_g^T@A2_g) and 4
    projection matmuls produce out[dst, 64] with no DVE reduce tree.
  - Per-chunk software pipeline: PE work of chunk sc-1 interleaves with
    blends of chunk sc; gathers stream on GPSIMD across 4 SWDGE queues.
"""

import os
import sys
import numpy as np

sys.path.insert(0, "/opt/trn_rl_repo")

import concourse.bass as bass
import concourse.bacc as bacc
import concourse.tile as tile
from concourse import mybir
from concourse.masks import make_identity
from concourse.tile_rust import add_dep_helper
import ml_dtypes

F32 = mybir.dt.float32
BF16 = mybir.dt.bfloat16
I16 = mybir.dt.int16
ALU = mybir.AluOpType


# --- patch: make Tile's DMASW lane assignment respect SWDGE queue_num.
import concourse.tile_sem_assignment as _tsa


def _patched_assign_tick(self, inst):
    import concourse.bass_isa as bass_isa_
    engine = inst.engine
    if (isinstance(inst, _tsa.DMAInst)
            and not isinstance(inst, bass_isa_.UserSyncedRemoteDMADescs)
            and engine == mybir.EngineType.Pool):
        q = getattr(inst, "queue_num", 0) or 0
        tog = getattr(self, "_gat_q_toggle", None)
        if tog is None:
            tog = self._gat_q_toggle = [0, 0, 0, 0]
        lane = q + 4 * tog[q]
        tog[q] ^= 1
        self.next_sw_dma_idx = lane
    return _tsa.TileClockTick._orig_assign_tick(self, inst)


if not hasattr(_tsa.TileClockTick, "_orig_assign_tick"):
    _tsa.TileClockTick._orig_assign_tick = _tsa.TileClockTick._assign_tick
    _tsa.TileClockTick._assign_tick = _patched_assign_tick

# ---------------------------------------------------------------- config

N = 50000
DEG = 16
DIN = 128
DOUT = 64
H = 4
CORES = 8
NROW = 25088        # paired rows: row r covers nodes r and r+NROW
NALL = 2 * NROW     # 50176 padded nodes
ROWE = 256          # bf16 elems per row (512B): h_lo 128 | h_hi-h_lo 128


class Cfg:
    def __init__(self, gdt=BF16):
        self.n_own = 6272
        self.T = 49
        self.S = 7
        self.NCH = 7
        self.gdt = BF16


def full_cfg(gdt=BF16):
    return Cfg()


# ---------------------------------------------------------------- sort network

def batcher_stages(n=16):
    stages = []
    p = 1
    while p < n:
        k = p
        while k >= 1:
            stage = []
            for j in range(k % p, n - k, 2 * k):
                for i in range(min(k, n - j - k)):
                    if (i + j) // (p * 2) == (i + j + k) // (p * 2):
                        stage.append((i + j, i + j + k))
            stages.append((k, stage))
            k //= 2
        p *= 2
    return stages


def group_lo(los):
    los = sorted(los)
    n = len(los)
    if n == 1:
        return los[0], [[1, 1], [1, 1]]
    d = [los[i + 1] - los[i] for i in range(n - 1)]
    r = 1
    while r < n and d[r - 1] == d[0]:
        r += 1
    istride = d[0]
    if r == n:
        return los[0], [[istride * n, 1], [istride, n]]
    assert n % r == 0, (los,)
    ostride = los[r] - los[0]
    for b in range(n // r):
        for i in range(r):
            assert los[b * r + i] == los[0] + b * ostride + i * istride, (los,)
    return los[0], [[ostride, n // r], [istride, r]]


SORT_STAGES = [(k, group_lo([lo for lo, _ in st])) for (k, st) in batcher_stages(16)]

# per-stage uncovered positions (copythrough in the ping-pong sort), as
# (lo, dims) AP fragments over the 16-wide neighbor axis
SORT_UNC = {
    2: (0, [[4, 4], [3, 2]]),
    4: (0, [[8, 2], [6, 2], [1, 2]]),
    5: (0, [[8, 2], [7, 2]]),
    7: (0, [[12, 2], [1, 4]]),
    8: (0, [[14, 2], [1, 2]]),
    9: (0, [[15, 2]]),
}


# ---------------------------------------------------------------- AP helper

def sub_ap(base_ap, off, dims):
    return bass.AP(
        tensor=base_ap.tensor,
        offset=base_ap.offset + off,
        ap=[list(base_ap.ap[0])] + [list(d) for d in dims],
    )


# ---------------------------------------------------------------- program

def build_program(cfg, num_devices=CORES):
    nc = bacc.Bacc("TRN2", target_bir_lowering=False, debug=False,
                   num_devices=num_devices,
                   dynamic_dma_scratch_size=int(os.environ.get("GAT_DMA_SCRATCH", 65536)),
                   num_swdge_queues=4)
    T, S, NCH = cfg.T, cfg.S, cfg.NCH
    W = S * 64
    NCK = NALL // 128   # 392 chunks for the P pass

    # ---- DRAM tensors
    h_tab = nc.dram_tensor("h_tab", [NROW, ROWE], BF16, kind="ExternalInput").ap()
    fc_wT = nc.dram_tensor("fc_wT", [H, DIN, DOUT], BF16, kind="ExternalInput").ap()
    pq_d = nc.dram_tensor("pq448", [128, T * 64], F32, kind="ExternalInput").ap()
    idx_d = nc.dram_tensor("idxP", [128, T * 128], I16, kind="ExternalInput").ap()
    selm_d = nc.dram_tensor("selm", [128, T * DEG * 8], BF16, kind="ExternalInput").ap()
    wq_d = nc.dram_tensor("wq448", [128, T * 64], F32, kind="ExternalInput").ap()
    kinv_d = nc.dram_tensor("kinv448", [128, W], F32, kind="ExternalInput").ap()
    ws_d = nc.dram_tensor("ws448", [128, W], F32, kind="ExternalInput").ap()
    smask_d = nc.dram_tensor("scanmask", [128, W], F32, kind="ExternalInput").ap()
    rh_d = nc.dram_tensor("rhrep", [64, H * 128], BF16, kind="ExternalInput").ap()
    blk_d = nc.dram_tensor("blkmask", [128, H * 128], BF16, kind="ExternalInput").ap()
    out_d = nc.dram_tensor("out", [cfg.n_own, DOUT], F32, kind="ExternalOutput").ap()

    from contextlib import ExitStack
    with tile.TileContext(nc) as tc, ExitStack() as ctx:
        singles = ctx.enter_context(tc.tile_pool(name="singles", bufs=1))

        kinv_sb = singles.tile([128, W], F32)
        ws_sb = singles.tile([128, W], F32)
        smask_sb = singles.tile([128, W], F32)
        selm_sb = singles.tile([128, T * DEG * 8], BF16)
        fcwT_sb = singles.tile([128, H * DOUT], BF16)
        ident = singles.tile([128, 128], BF16)
        rh_sb = singles.tile([64, H * 128], BF16)
        blk_sb = singles.tile([128, H * 128], BF16)
        zero_sb = singles.tile([128, W], F32)
        nc.vector.memset(zero_sb[:], 0.0)
        nc.sync.dma_start(out=rh_sb[:], in_=rh_d)
        nc.sync.dma_start(out=blk_sb[:], in_=blk_d)

        nc.scalar.dma_start(out=kinv_sb[:], in_=kinv_d)
        nc.scalar.dma_start(out=ws_sb[:], in_=ws_d)
        nc.scalar.dma_start(out=smask_sb[:], in_=smask_d)
        nc.scalar.dma_start(out=selm_sb[:], in_=selm_d)
        for hh in range(H):
            nc.sync.dma_start(out=fcwT_sb[:, hh * DOUT:(hh + 1) * DOUT], in_=fc_wT[hh])
        make_identity(nc, ident[:])

        # ---- stage B pools
        ga_pool = ctx.enter_context(tc.tile_pool(name="ga", bufs=12))
        arep_pool = ctx.enter_context(tc.tile_pool(name="arep", bufs=2))
        ck_pool = ctx.enter_context(tc.tile_pool(name="ck", bufs=2))
        sc_pool = ctx.enter_context(tc.tile_pool(name="cks", bufs=1))
        wqp = ctx.enter_context(tc.tile_pool(name="wqp", bufs=4))
        idx_pool = ctx.enter_context(tc.tile_pool(name="idx", bufs=2))
        m_pool = ctx.enter_context(tc.tile_pool(name="m", bufs=2))
        mt_pool = ctx.enter_context(tc.tile_pool(name="mt", bufs=4))
        ob_pool = ctx.enter_context(tc.tile_pool(name="ob", bufs=2))
        tr_pool = ctx.enter_context(tc.tile_pool(name="tr", bufs=1, space="PSUM"))
        apr_pool = ctx.enter_context(tc.tile_pool(name="apr", bufs=2, space="PSUM"))
        mtp_pool = ctx.enter_context(tc.tile_pool(name="mtp", bufs=2, space="PSUM"))
        asb_pool = ctx.enter_context(tc.tile_pool(name="asb", bufs=4))
        pr_pool = ctx.enter_context(tc.tile_pool(name="pr", bufs=2, space="PSUM"))

        zs_t = sc_pool.tile([128, W], F32, tag="zs")
        A_t = sc_pool.tile([128, W], F32, tag="A")
        B_t = sc_pool.tile([128, W], F32, tag="B")
        C_t = sc_pool.tile([128, W], F32, tag="C")
        ts4_t = sc_pool.tile([128, S * 4], F32, tag="ts4")

        def do_fma(prev, tl):
            """alpha spread via replication matmul + FMA and projection on PE."""
            gas_p, z_p, sc_p = prev
            t_glob = sc_p * S + tl
            ga_p = gas_p[tl]
            ab = arep_pool.tile([128, 64], BF16, tag="arep")
            nc.scalar.copy(out=ab[:], in_=z_p[:, tl * 64:(tl + 1) * 64])
            trA = tr_pool.tile([64, 128], BF16, tag="trA")
            nc.tensor.transpose(out=trA[:], in_=ab[:], identity=ident[:])
            aT = mt_pool.tile([64, 128], BF16, tag="mt")
            nc.scalar.copy(out=aT[:], in_=trA[:])
            # Aprec[p=(j,d), (h, dstl)] = alphaT[(h, d(p)), dstl]
            apr = apr_pool.tile([128, H * 128], F32, tag="apr")
            for hh in range(H):
                nc.tensor.matmul(out=apr[:, hh * 128:(hh + 1) * 128],
                                 lhsT=rh_sb[:, hh * 128:(hh + 1) * 128],
                                 rhs=aT[:], start=True, stop=True)
            # block-diag mask (zero where j(p) != dstl%8), f32->bf16,
            # permuted to (g, h, j) so each group's 32 rhs cols are contiguous
            A_sb = asb_pool.tile([128, H * 128], BF16, tag="asb")
            nc.vector.tensor_mul(
                out=sub_ap(A_sb[:], 0, [[8, H], [32, 16], [1, 8]]),
                in0=sub_ap(apr[:], 0, [[128, H], [8, 16], [1, 8]]),
                in1=sub_ap(blk_sb[:], 0, [[128, H], [8, 16], [1, 8]]))
            # A2 = A * s (selects hi-half via the delta columns)
            A2_sb = asb_pool.tile([128, H * 128], BF16, tag="asb")
            nc.vector.tensor_mul(
                out=sub_ap(A2_sb[:], 0, [[32, 16], [8, H], [1, 8]]),
                in0=sub_ap(A_sb[:], 0, [[32, 16], [8, H], [1, 8]]),
                in1=sub_ap(selm_sb[:], t_glob * DEG * 8,
                           [[8, 16], [0, H], [1, 8]]))
            # mT[f, g*32+h*8+j] = lo_g^T @ A_g + delta_g^T @ A2_g  (blend folded)
            mT = mtp_pool.tile([128, 512], F32, tag="mtp")
            for g in range(16):
                nc.tensor.matmul(
                    out=mT[:, g * 32:(g + 1) * 32],
                    lhsT=ga_p[:, g * ROWE:g * ROWE + DIN],
                    rhs=A_sb[:, g * 32:(g + 1) * 32],
                    start=True, stop=False)
                nc.tensor.matmul(
                    out=mT[:, g * 32:(g + 1) * 32],
                    lhsT=ga_p[:, g * ROWE + DIN:(g + 1) * ROWE],
                    rhs=A2_sb[:, g * 32:(g + 1) * 32],
                    start=False, stop=True)
            mTs = m_pool.tile([128, 512], BF16, tag="m")
            nc.scalar.copy(
                out=sub_ap(mTs[:], 0, [[128, H], [8, 16], [1, 8]]),
                in_=sub_ap(mT[:], 0, [[8, H], [32, 16], [1, 8]]))
            proj = pr_pool.tile([128, DOUT], F32, tag="pr")
            for hh in range(H):
                nc.tensor.matmul(out=proj[:],
                                 lhsT=mTs[:, hh * 128:(hh + 1) * 128],
                                 rhs=fcwT_sb[:, hh * DOUT:(hh + 1) * DOUT],
                                 start=(hh == 0), stop=(hh == H - 1))
            osb = ob_pool.tile([128, DOUT], F32, tag="ob")
            nc.scalar.copy(out=osb[:], in_=proj[:])
            nc.sync.dma_start(out=out_d[t_glob * 128:(t_glob + 1) * 128, :],
                              in_=osb[:])

        prev = None
        for sc in range(NCH):
            idx_sb = idx_pool.tile([128, S * 128], I16, tag="idx")
            nc.sync.dma_start(out=idx_sb[:],
                              in_=idx_d[:, sc * S * 128:(sc + 1) * S * 128])
            wq_sb = wqp.tile([128, W], F32, tag="wq")
            nc.sync.dma_start(out=wq_sb[:], in_=wq_d[:, sc * W:(sc + 1) * W])
            pq_sb = wqp.tile([128, W], F32, tag="pq")
            nc.sync.dma_start(out=pq_sb[:], in_=pq_d[:, sc * W:(sc + 1) * W])
            pt = ck_pool.tile([128, W], F32, tag="pt")

            gas = []
            for tl in range(S):
                t_glob = sc * S + tl
                ga = ga_pool.tile([128, DEG * ROWE], BF16, tag="ga")
                for gh in range(2):
                    o3 = bass.AP(tensor=ga[:].tensor,
                                 offset=ga[:].offset + gh * 8 * ROWE,
                                 ap=[list(ga[:].ap[0]), [ROWE, 8], [1, ROWE]])
                    g = nc.gpsimd.dma_gather(
                        out_ap=o3, in_ap=h_tab,
                        idxs_ap=idx_sb[:, tl * 128 + gh * 64:tl * 128 + (gh + 1) * 64],
                        num_idxs=8 * 128, num_idxs_reg=8 * 128,
                        elem_size=ROWE, single_packet=False,
                        queue_num=(2 * t_glob + gh) % 4)
                gas.append(ga)

            # ---- chunk ops: logits, sort, entmax -> alpha (in place in pt)
            z = pt
            nc.vector.scalar_tensor_tensor(out=z[:], in0=pq_sb[:], scalar=0.01,
                                           in1=pq_sb[:], op0=ALU.mult, op1=ALU.max)
            nc.vector.tensor_add(out=z[:], in0=z[:], in1=wq_sb[:])

            # sort descending into zs (C as CE scratch)
            nc.scalar.copy(out=zs_t[:], in_=z[:])
            for k, (lo0, dims) in SORT_STAGES:
                ap_dims = [[16, S * 4]] + [[d[0], d[1]] for d in dims]
                a_ap = sub_ap(zs_t[:], lo0, ap_dims)
                b_ap = sub_ap(zs_t[:], lo0 + k, ap_dims)
                t_ap = sub_ap(C_t[:], lo0, ap_dims)
                nc.vector.tensor_tensor(out=t_ap, in0=a_ap, in1=b_ap, op=ALU.min)
                nc.vector.tensor_tensor(out=a_ap, in0=a_ap, in1=b_ap, op=ALU.max)
                nc.vector.tensor_tensor(out=b_ap, in0=t_ap, in1=t_ap, op=ALU.max)

            # segmented cumsums via scan: state = mask*state + x
            nc.vector.tensor_tensor_scan(out=A_t[:], data0=smask_sb[:], data1=zs_t[:],
                                         initial=0.0, op0=ALU.mult, op1=ALU.add)
            nc.vector.tensor_mul(out=C_t[:], in0=zs_t[:], in1=zs_t[:])
            nc.vector.tensor_tensor_scan(out=B_t[:], data0=smask_sb[:], data1=C_t[:],
                                         initial=0.0, op0=ALU.mult, op1=ALU.add)

            # entmax threshold
            nc.vector.tensor_mul(out=C_t[:], in0=A_t[:], in1=A_t[:])
            nc.vector.tensor_mul(out=C_t[:], in0=C_t[:], in1=kinv_sb[:])
            nc.vector.tensor_sub(out=C_t[:], in0=B_t[:], in1=C_t[:])      # ss
            nc.vector.tensor_mul(out=B_t[:], in0=C_t[:], in1=kinv_sb[:])
            nc.vector.tensor_sub(out=B_t[:], in0=kinv_sb[:], in1=B_t[:])  # (1-ss)/k
            nc.vector.tensor_tensor(out=B_t[:], in0=B_t[:], in1=zero_sb[:], op=ALU.max)
            nc.scalar.sqrt(out=B_t[:], in_=B_t[:])
            nc.vector.tensor_mul(out=A_t[:], in0=A_t[:], in1=kinv_sb[:])  # mean
            nc.vector.tensor_sub(out=A_t[:], in0=A_t[:], in1=B_t[:])      # tau

            nc.vector.tensor_tensor(out=C_t[:], in0=A_t[:], in1=zs_t[:], op=ALU.is_le)
            e15 = [[64, S], [16, 4], [1, DEG - 1]]
            nc.vector.tensor_sub(out=sub_ap(B_t[:], 0, e15),
                                 in0=sub_ap(C_t[:], 0, e15),
                                 in1=sub_ap(C_t[:], 1, e15))
            nc.scalar.copy(out=sub_ap(B_t[:], DEG - 1, [[64, S], [16, 4], [1, 1]]),
                           in_=sub_ap(C_t[:], DEG - 1, [[64, S], [16, 4], [1, 1]]))
            nc.vector.tensor_mul(out=B_t[:], in0=B_t[:], in1=A_t[:])
            nc.vector.tensor_reduce(
                out=sub_ap(ts4_t[:], 0, [[4, S], [1, 4]]),
                in_=sub_ap(B_t[:], 0, [[64, S], [16, 4], [1, DEG]]),
                axis=mybir.AxisListType.X, op=ALU.add)
            nc.scalar.copy(
                out=sub_ap(C_t[:], 0, [[64, S], [16, 4], [1, 16]]),
                in_=sub_ap(ts4_t[:], 0, [[4, S], [1, 4], [0, 16]]))   # tau* rep
            nc.vector.tensor_sub(out=z[:], in0=z[:], in1=C_t[:])
            nc.vector.tensor_tensor(out=z[:], in0=z[:], in1=zero_sb[:], op=ALU.max)
            nc.vector.tensor_mul(out=z[:], in0=z[:], in1=z[:])
            nc.vector.tensor_mul(out=z[:], in0=z[:], in1=ws_sb[:])

            for tl in range(S):
                if prev is not None:
                    do_fma(prev, tl)

            prev = (gas, z, sc)

        for tl in range(S):
            do_fma(prev, tl)

    nc.compile()
    return nc


# ---------------------------------------------------------------- host prep

def softmax_np(x):
    e = np.exp(x - np.max(x))
    return e / e.sum()


def host_prep(cfg, h, src, w, fc_w, attn_w, head_weights, n_cores, n_total=N):
    n_own_real = n_total // n_cores
    T, S = cfg.T, cfg.S
    W = S * 64

    h_pad = np.zeros((NALL, DIN), np.float32)
    h_pad[:n_total] = h
    hq = h_pad.astype(ml_dtypes.bfloat16)

    # paired gather table: row r = [h_r | h_{r+NROW} - h_r]
    h_tab = np.zeros((NROW, ROWE), ml_dtypes.bfloat16)
    h_tab[:, 0:DIN] = hq[:NROW]
    h_tab[:, DIN:2 * DIN] = (hq[NROW:].astype(np.float32)
                             - hq[:NROW].astype(np.float32)).astype(ml_dtypes.bfloat16)

    fc_wT = np.ascontiguousarray(np.transpose(fc_w, (0, 2, 1))).astype(ml_dtypes.bfloat16)

    # dense per-node logit projections (f32, host): P = h.U_src, Q = h.U_dst
    fc_w32 = fc_w.astype(np.float32)
    U_src = np.stack([fc_w32[hh].T @ attn_w[hh, :DOUT] for hh in range(H)], 1)
    U_dst = np.stack([fc_w32[hh].T @ attn_w[hh, DOUT:] for hh in range(H)], 1)
    P_all = h_pad @ U_src          # [NALL, H]
    Q_all = h_pad @ U_dst          # [NALL, H]

    ws = softmax_np(head_weights.astype(np.float32))
    hcol = np.arange(W) % 64                       # within-tile col = h*16+d
    h_of = hcol // DEG
    d_of = hcol % DEG
    kinv448 = np.tile((1.0 / (d_of + 1.0))[None, :], (128, 1)).astype(np.float32)
    ws448 = np.tile(ws[h_of][None, :], (128, 1)).astype(np.float32)
    smask = np.tile((d_of != 0).astype(np.float32)[None, :], (128, 1))

    src2d = src.reshape(n_total, DEG).astype(np.int64)
    w2d = w.reshape(n_total, DEG).astype(np.float32)

    i1024 = np.arange(1024)
    dh_ = i1024 // 128
    ph_ = i1024 % 128

    q64 = np.arange(64)
    p128 = np.arange(128)
    rh = np.zeros((64, H * 128), np.float32)
    for hh in range(H):
        rh[:, hh * 128:(hh + 1) * 128] = (
            (q64[:, None] // 16 == hh) & (q64[:, None] % 16 == p128[None, :] % 16))
    rhrep = rh.astype(ml_dtypes.bfloat16)
    blk = np.zeros((128, H * 128), np.float32)
    for hh in range(H):
        blk[:, hh * 128:(hh + 1) * 128] = (p128[None, :] % 8 == p128[:, None] // 16)
    blkmask = blk.astype(ml_dtypes.bfloat16)

    in_maps = []
    for c in range(n_cores):
        lo = c * n_own_real
        hi = lo + n_own_real
        own_src = np.zeros((cfg.n_own, DEG), np.int64)
        own_src[:n_own_real] = src2d[lo:hi]
        own_w = np.zeros((cfg.n_own, DEG), np.float32)
        own_w[:n_own_real] = 0.5 * w2d[lo:hi]

        sel = (own_src >= NROW)
        row = np.where(sel, own_src - NROW, own_src)

        # edge-major slots: out partition p = (dst%8)*16 + d, row v = dst//8
        idxP = np.zeros((128, T * 128), np.int16)
        for t in range(T):
            for gh in range(2):
                dstl = (8 * gh + dh_) * 8 + ph_ // 16
                vals = row[t * 128 + dstl, ph_ % 16].astype(np.int16)
                pat = np.zeros((16, 64), np.int16)
                pat[i1024 % 16, i1024 // 16] = vals
                idxP[:, t * 128 + gh * 64:t * 128 + (gh + 1) * 64] = \
                    np.tile(pat, (8, 1))

        sf = sel.astype(np.float32)                       # [n_own, DEG]
        jj = np.arange(128) // 16
        dd16 = np.arange(128) % 16
        sfr = sf.reshape(T, 16, 8, DEG)
        selE = sfr[:, :, jj, dd16].transpose(2, 0, 1)     # [128, T, 16] edge-major
        selm = np.ascontiguousarray(
            np.repeat(selE.reshape(128, T * DEG)[:, :, None], 8, axis=2)
            .reshape(128, T * DEG * 8)).astype(ml_dtypes.bfloat16)

        # pq448: [p, t*64 + h*16 + d] = 0.5*(P[src] + Q[dst])
        own_nodes = np.zeros(cfg.n_own, np.int64)
        own_nodes[:n_own_real] = np.arange(lo, hi)
        pq = 0.5 * (P_all[own_src] + Q_all[own_nodes][:, None, :])  # [n_own, DEG, H]
        pq = pq.transpose(0, 2, 1).reshape(T, 128, 64).transpose(1, 0, 2)
        pq448 = np.ascontiguousarray(pq.reshape(128, T * 64)).astype(np.float32)

        # wq448: [p, t*64 + h*16 + d] = 0.5*w[node(t,p), d]
        w3 = own_w.reshape(T, 128, DEG).transpose(1, 0, 2)   # [128, T, DEG]
        wq448 = np.tile(w3[:, :, None, :], (1, 1, H, 1)).reshape(128, T * 64)
        wq448 = np.ascontiguousarray(wq448).astype(np.float32)

        in_maps.append({
            "h_tab": h_tab, "fc_wT": fc_wT, "pq448": pq448,
            "idxP": idxP, "selm": selm, "rhrep": rhrep, "blkmask": blkmask,
            "wq448": wq448, "kinv448": kinv448, "ws448": ws448,
            "scanmask": smask,
        })
    return in_maps


# ---------------------------------------------------------------- entry point

_PROG_CACHE = {}


def kernel(h, src, w, fc_w, attn_w, head_weights):
    h = np.asarray(h, np.float32)
    src = np.asarray(src)
    w = np.asarray(w, np.float32)
    fc_w = np.asarray(fc_w, np.float32)
    attn_w = np.asarray(attn_w, np.float32)
    head_weights = np.asarray(head_weights, np.float32)

    cfg = full_cfg()
    key = ("full",)
    if key not in _PROG_CACHE:
        _PROG_CACHE[key] = build_program(cfg, num_devices=CORES)
    nc = _PROG_CACHE[key]

    in_maps = host_prep(cfg, h, src, w, fc_w, attn_w, head_weights, CORES)

    from concourse.bass_utils import run_bass_kernel_spmd
    res = run_bass_kernel_spmd(nc, in_maps, core_ids=list(range(CORES)))

    n_own_real = N // CORES
    out = np.concatenate(
        [res.results[c]["out"][:n_own_real] for c in range(CORES)], axis=0)
    return out.astype(np.float32)



# revision 35
# speedup vs baseline: 1.0752x; 1.0048x over previous
"""Multi-head GAT layer (entmax15 attention over fixed-degree mailbox) on 8 trn2 cores.

Strategy (per core, dst-node sharded; full inputs in, full output out):
  - Each core owns N/8 destination nodes and their DEG=16 incoming edges.
  - ONE dma_gather per 128-node tile from a paired-row table in EDGE-major
    slot order (partition p = (dst%8)*16 + d, row = dst//8): row r =
    [h_r | h_{r+25088}-h_r] (512B), so int16 indices cover all 50k nodes.
  - Logit biases P_src = h.U_src and Q_dst = h.U_dst are dense per-node
    projections folded on the HOST into pq448 (like wq448), so no P table,
    no stage A, and rows carry only features.
  - The lo/hi select is folded into the FMA: since s is 0/1,
    sum((lo+s*d)*a) = lo^T@A + d^T@(s*A), so no DVE blend exists at all;
    A2 = A*s costs one small DVE multiply per tile.
  - entmax15 (dst-major, chunk-wide): Batcher sort network + segmented
    cumsum via tensor_tensor_scan + threshold, all on DVE.
  - FMA + projection run on the TENSOR engine: alpha [dst,(h,d)] is
    PE-transposed, spread along j=dst%8 by a constant replication matmul
    (Aprec[p,(h,dst)] = alphaT[(h,d(p)),dst]), block-diag-masked on DVE,
    then 32 PSUM-accumulating matmuls (lo_g^T@A_g + d_g^T@A2_g) and 4
    projection matmuls produce out[dst, 64] with no DVE reduce tree.
  - Per-chunk software pipeline: PE work of chunk sc-1 interleaves with
    blends of chunk sc; gathers stream on GPSIMD across 4 SWDGE queues.
"""

import os
import sys
import numpy as np

sys.path.insert(0, "/opt/trn_rl_repo")

import concourse.bass as bass
import concourse.bacc as bacc
import concourse.tile as tile
from concourse import mybir
from concourse.masks import make_identity
from concourse.tile_rust import add_dep_helper
import ml_dtypes

F32 = mybir.dt.float32
BF16 = mybir.dt.bfloat16
I16 = mybir.dt.int16
ALU = mybir.AluOpType


# --- patch: make Tile's DMASW lane assignment respect SWDGE queue_num.
import concourse.tile_sem_assignment as _tsa


def _patched_assign_tick(self, inst):
    import concourse.bass_isa as bass_isa_
    engine = inst.engine
    if (isinstance(inst, _tsa.DMAInst)
            and not isinstance(inst, bass_isa_.UserSyncedRemoteDMADescs)
            and engine == mybir.EngineType.Pool):
        q = getattr(inst, "queue_num", 0) or 0
        tog = getattr(self, "_gat_q_toggle", None)
        if tog is None:
            tog = self._gat_q_toggle = [0, 0, 0, 0]
        lane = q + 4 * tog[q]
        tog[q] ^= 1
        self.next_sw_dma_idx = lane
    return _tsa.TileClockTick._orig_assign_tick(self, inst)


if not hasattr(_tsa.TileClockTick, "_orig_assign_tick"):
    _tsa.TileClockTick._orig_assign_tick = _tsa.TileClockTick._assign_tick
    _tsa.TileClockTick._assign_tick = _patched_assign_tick

# ---------------------------------------------------------------- config

N = 50000
DEG = 16
DIN = 128
DOUT = 64
H = 4
CORES = 8
NROW = 25088        # paired rows: row r covers nodes r and r+NROW
NALL = 2 * NROW     # 50176 padded nodes
ROWE = 256          # bf16 elems per row (512B): h_lo 128 | h_hi-h_lo 128


class Cfg:
    def __init__(self, gdt=BF16):
        self.n_own = 6272
        self.T = 49
        self.S = 7
        self.NCH = 7
        self.gdt = BF16


def full_cfg(gdt=BF16):
    return Cfg()


# ---------------------------------------------------------------- sort network

def batcher_stages(n=16):
    stages = []
    p = 1
    while p < n:
        k = p
        while k >= 1:
            stage = []
            for j in range(k % p, n - k, 2 * k):
                for i in range(min(k, n - j - k)):
                    if (i + j) // (p * 2) == (i + j + k) // (p * 2):
                        stage.append((i + j, i + j + k))
            stages.append((k, stage))
            k //= 2
        p *= 2
    return stages


def group_lo(los):
    los = sorted(los)
    n = len(los)
    if n == 1:
        return los[0], [[1, 1], [1, 1]]
    d = [los[i + 1] - los[i] for i in range(n - 1)]
    r = 1
    while r < n and d[r - 1] == d[0]:
        r += 1
    istride = d[0]
    if r == n:
        return los[0], [[istride * n, 1], [istride, n]]
    assert n % r == 0, (los,)
    ostride = los[r] - los[0]
    for b in range(n // r):
        for i in range(r):
            assert los[b * r + i] == los[0] + b * ostride + i * istride, (los,)
    return los[0], [[ostride, n // r], [istride, r]]


SORT_STAGES = [(k, group_lo([lo for lo, _ in st])) for (k, st) in batcher_stages(16)]

# per-stage uncovered positions (copythrough in the ping-pong sort), as
# (lo, dims) AP fragments over the 16-wide neighbor axis
SORT_UNC = {
    2: (0, [[4, 4], [3, 2]]),
    4: (0, [[8, 2], [6, 2], [1, 2]]),
    5: (0, [[8, 2], [7, 2]]),
    7: (0, [[12, 2], [1, 4]]),
    8: (0, [[14, 2], [1, 2]]),
    9: (0, [[15, 2]]),
}


# ---------------------------------------------------------------- AP helper

def sub_ap(base_ap, off, dims):
    return bass.AP(
        tensor=base_ap.tensor,
        offset=base_ap.offset + off,
        ap=[list(base_ap.ap[0])] + [list(d) for d in dims],
    )


# ---------------------------------------------------------------- program

def build_program(cfg, num_devices=CORES):
    nc = bacc.Bacc("TRN2", target_bir_lowering=False, debug=False,
                   num_devices=num_devices,
                   dynamic_dma_scratch_size=int(os.environ.get("GAT_DMA_SCRATCH", 65536)),
                   num_swdge_queues=4)
    T, S, NCH = cfg.T, cfg.S, cfg.NCH
    W = S * 64
    NCK = NALL // 128   # 392 chunks for the P pass

    # ---- DRAM tensors
    h_tab = nc.dram_tensor("h_tab", [NROW, ROWE], BF16, kind="ExternalInput").ap()
    fc_wT = nc.dram_tensor("fc_wT", [H, DIN, DOUT], BF16, kind="ExternalInput").ap()
    pq_d = nc.dram_tensor("pq448", [128, T * 64], F32, kind="ExternalInput").ap()
    idx_d = nc.dram_tensor("idxP", [128, T * 128], I16, kind="ExternalInput").ap()
    selm_d = nc.dram_tensor("selm", [128, T * DEG * 8], BF16, kind="ExternalInput").ap()
    wq_d = nc.dram_tensor("wq448", [128, T * 64], F32, kind="ExternalInput").ap()
    kinv_d = nc.dram_tensor("kinv448", [128, W], F32, kind="ExternalInput").ap()
    ws_d = nc.dram_tensor("ws448", [128, W], F32, kind="ExternalInput").ap()
    smask_d = nc.dram_tensor("scanmask", [128, W], F32, kind="ExternalInput").ap()
    rh_d = nc.dram_tensor("rhrep", [64, H * 128], BF16, kind="ExternalInput").ap()
    blk_d = nc.dram_tensor("blkmask", [128, H * 128], BF16, kind="ExternalInput").ap()
    out_d = nc.dram_tensor("out", [cfg.n_own, DOUT], F32, kind="ExternalOutput").ap()

    from contextlib import ExitStack
    with tile.TileContext(nc) as tc, ExitStack() as ctx:
        singles = ctx.enter_context(tc.tile_pool(name="singles", bufs=1))

        kinv_sb = singles.tile([128, W], F32)
        ws_sb = singles.tile([128, W], F32)
        smask_sb = singles.tile([128, W], F32)
        selm_sb = singles.tile([128, T * DEG * 8], BF16)
        fcwT_sb = singles.tile([128, H * DOUT], BF16)
        ident = singles.tile([128, 128], BF16)
        rh_sb = singles.tile([64, H * 128], BF16)
        blk_sb = singles.tile([128, H * 128], BF16)
        zero_sb = singles.tile([128, W], F32)
        nc.vector.memset(zero_sb[:], 0.0)
        nc.sync.dma_start(out=rh_sb[:], in_=rh_d)
        nc.sync.dma_start(out=blk_sb[:], in_=blk_d)

        nc.sync.dma_start(out=kinv_sb[:], in_=kinv_d)
        nc.sync.dma_start(out=ws_sb[:], in_=ws_d)
        nc.sync.dma_start(out=smask_sb[:], in_=smask_d)
        nc.sync.dma_start(out=selm_sb[:], in_=selm_d)
        for hh in range(H):
            nc.sync.dma_start(out=fcwT_sb[:, hh * DOUT:(hh + 1) * DOUT], in_=fc_wT[hh])
        make_identity(nc, ident[:])

        # ---- stage B pools
        ga_pool = ctx.enter_context(tc.tile_pool(name="ga", bufs=12))
        arep_pool = ctx.enter_context(tc.tile_pool(name="arep", bufs=2))
        ck_pool = ctx.enter_context(tc.tile_pool(name="ck", bufs=2))
        sc_pool = ctx.enter_context(tc.tile_pool(name="cks", bufs=1))
        wqp = ctx.enter_context(tc.tile_pool(name="wqp", bufs=4))
        idx_pool = ctx.enter_context(tc.tile_pool(name="idx", bufs=2))
        m_pool = ctx.enter_context(tc.tile_pool(name="m", bufs=2))
        mt_pool = ctx.enter_context(tc.tile_pool(name="mt", bufs=4))
        ob_pool = ctx.enter_context(tc.tile_pool(name="ob", bufs=2))
        tr_pool = ctx.enter_context(tc.tile_pool(name="tr", bufs=1, space="PSUM"))
        apr_pool = ctx.enter_context(tc.tile_pool(name="apr", bufs=2, space="PSUM"))
        mtp_pool = ctx.enter_context(tc.tile_pool(name="mtp", bufs=2, space="PSUM"))
        asb_pool = ctx.enter_context(tc.tile_pool(name="asb", bufs=4))
        pr_pool = ctx.enter_context(tc.tile_pool(name="pr", bufs=2, space="PSUM"))

        zs_t = sc_pool.tile([128, W], F32, tag="zs")
        A_t = sc_pool.tile([128, W], F32, tag="A")
        B_t = sc_pool.tile([128, W], F32, tag="B")
        C_t = sc_pool.tile([128, W], F32, tag="C")
        ts4_t = sc_pool.tile([128, S * 4], F32, tag="ts4")

        def do_fma(prev, tl):
            """alpha spread via replication matmul + FMA and projection on PE."""
            gas_p, z_p, sc_p = prev
            t_glob = sc_p * S + tl
            ga_p = gas_p[tl]
            ab = arep_pool.tile([128, 64], BF16, tag="arep")
            nc.scalar.copy(out=ab[:], in_=z_p[:, tl * 64:(tl + 1) * 64])
            trA = tr_pool.tile([64, 128], BF16, tag="trA")
            nc.tensor.transpose(out=trA[:], in_=ab[:], identity=ident[:])
            aT = mt_pool.tile([64, 128], BF16, tag="mt")
            nc.scalar.copy(out=aT[:], in_=trA[:])
            # Aprec[p=(j,d), (h, dstl)] = alphaT[(h, d(p)), dstl]
            apr = apr_pool.tile([128, H * 128], F32, tag="apr")
            for hh in range(H):
                nc.tensor.matmul(out=apr[:, hh * 128:(hh + 1) * 128],
                                 lhsT=rh_sb[:, hh * 128:(hh + 1) * 128],
                                 rhs=aT[:], start=True, stop=True)
            # block-diag mask (zero where j(p) != dstl%8), f32->bf16,
            # permuted to (g, h, j) so each group's 32 rhs cols are contiguous
            A_sb = asb_pool.tile([128, H * 128], BF16, tag="asb")
            nc.vector.tensor_mul(
                out=sub_ap(A_sb[:], 0, [[8, H], [32, 16], [1, 8]]),
                in0=sub_ap(apr[:], 0, [[128, H], [8, 16], [1, 8]]),
                in1=sub_ap(blk_sb[:], 0, [[128, H], [8, 16], [1, 8]]))
            # A2 = A * s (selects hi-half via the delta columns)
            A2_sb = asb_pool.tile([128, H * 128], BF16, tag="asb")
            nc.vector.tensor_mul(
                out=sub_ap(A2_sb[:], 0, [[32, 16], [8, H], [1, 8]]),
                in0=sub_ap(A_sb[:], 0, [[32, 16], [8, H], [1, 8]]),
                in1=sub_ap(selm_sb[:], t_glob * DEG * 8,
                           [[8, 16], [0, H], [1, 8]]))
            # mT[f, g*32+h*8+j] = lo_g^T @ A_g + delta_g^T @ A2_g  (blend folded)
            mT = mtp_pool.tile([128, 512], F32, tag="mtp")
            for g in range(16):
                nc.tensor.matmul(
                    out=mT[:, g * 32:(g + 1) * 32],
                    lhsT=ga_p[:, g * ROWE:g * ROWE + DIN],
                    rhs=A_sb[:, g * 32:(g + 1) * 32],
                    start=True, stop=False)
                nc.tensor.matmul(
                    out=mT[:, g * 32:(g + 1) * 32],
                    lhsT=ga_p[:, g * ROWE + DIN:(g + 1) * ROWE],
                    rhs=A2_sb[:, g * 32:(g + 1) * 32],
                    start=False, stop=True)
            mTs = m_pool.tile([128, 512], BF16, tag="m")
            nc.scalar.copy(
                out=sub_ap(mTs[:], 0, [[128, H], [8, 16], [1, 8]]),
                in_=sub_ap(mT[:], 0, [[8, H], [32, 16], [1, 8]]))
            proj = pr_pool.tile([128, DOUT], F32, tag="pr")
            for hh in range(H):
                nc.tensor.matmul(out=proj[:],
                                 lhsT=mTs[:, hh * 128:(hh + 1) * 128],
                                 rhs=fcwT_sb[:, hh * DOUT:(hh + 1) * DOUT],
                                 start=(hh == 0), stop=(hh == H - 1))
            osb = ob_pool.tile([128, DOUT], F32, tag="ob")
            nc.scalar.copy(out=osb[:], in_=proj[:])
            nc.sync.dma_start(out=out_d[t_glob * 128:(t_glob + 1) * 128, :],
                              in_=osb[:])

        prev = None
        for sc in range(NCH):
            idx_sb = idx_pool.tile([128, S * 128], I16, tag="idx")
            nc.sync.dma_start(out=idx_sb[:],
                              in_=idx_d[:, sc * S * 128:(sc + 1) * S * 128])
            wq_sb = wqp.tile([128, W], F32, tag="wq")
            nc.sync.dma_start(out=wq_sb[:], in_=wq_d[:, sc * W:(sc + 1) * W])
            pq_sb = wqp.tile([128, W], F32, tag="pq")
            nc.sync.dma_start(out=pq_sb[:], in_=pq_d[:, sc * W:(sc + 1) * W])
            pt = ck_pool.tile([128, W], F32, tag="pt")

            gas = []
            for tl in range(S):
                t_glob = sc * S + tl
                ga = ga_pool.tile([128, DEG * ROWE], BF16, tag="ga")
                for gh in range(2):
                    o3 = bass.AP(tensor=ga[:].tensor,
                                 offset=ga[:].offset + gh * 8 * ROWE,
                                 ap=[list(ga[:].ap[0]), [ROWE, 8], [1, ROWE]])
                    g = nc.gpsimd.dma_gather(
                        out_ap=o3, in_ap=h_tab,
                        idxs_ap=idx_sb[:, tl * 128 + gh * 64:tl * 128 + (gh + 1) * 64],
                        num_idxs=8 * 128, num_idxs_reg=8 * 128,
                        elem_size=ROWE, single_packet=False,
                        queue_num=(2 * t_glob + gh) % 4)
                gas.append(ga)

            # ---- chunk ops: logits, sort, entmax -> alpha (in place in pt)
            z = pt
            nc.vector.scalar_tensor_tensor(out=z[:], in0=pq_sb[:], scalar=0.01,
                                           in1=pq_sb[:], op0=ALU.mult, op1=ALU.max)
            nc.vector.tensor_add(out=z[:], in0=z[:], in1=wq_sb[:])

            # sort descending into zs (C as CE scratch)
            nc.scalar.copy(out=zs_t[:], in_=z[:])
            for k, (lo0, dims) in SORT_STAGES:
                ap_dims = [[16, S * 4]] + [[d[0], d[1]] for d in dims]
                a_ap = sub_ap(zs_t[:], lo0, ap_dims)
                b_ap = sub_ap(zs_t[:], lo0 + k, ap_dims)
                t_ap = sub_ap(C_t[:], lo0, ap_dims)
                nc.vector.tensor_tensor(out=t_ap, in0=a_ap, in1=b_ap, op=ALU.min)
                nc.vector.tensor_tensor(out=a_ap, in0=a_ap, in1=b_ap, op=ALU.max)
                nc.vector.tensor_tensor(out=b_ap, in0=t_ap, in1=t_ap, op=ALU.max)

            # segmented cumsums via scan: state = mask*state + x
            nc.vector.tensor_tensor_scan(out=A_t[:], data0=smask_sb[:], data1=zs_t[:],
                                         initial=0.0, op0=ALU.mult, op1=ALU.add)
            nc.vector.tensor_mul(out=C_t[:], in0=zs_t[:], in1=zs_t[:])
            nc.vector.tensor_tensor_scan(out=B_t[:], data0=smask_sb[:], data1=C_t[:],
                                         initial=0.0, op0=ALU.mult, op1=ALU.add)

            # entmax threshold
            nc.vector.tensor_mul(out=C_t[:], in0=A_t[:], in1=A_t[:])
            nc.vector.tensor_mul(out=C_t[:], in0=C_t[:], in1=kinv_sb[:])
            nc.vector.tensor_sub(out=C_t[:], in0=B_t[:], in1=C_t[:])      # ss
            nc.vector.tensor_mul(out=B_t[:], in0=C_t[:], in1=kinv_sb[:])
            nc.vector.tensor_sub(out=B_t[:], in0=kinv_sb[:], in1=B_t[:])  # (1-ss)/k
            nc.vector.tensor_tensor(out=B_t[:], in0=B_t[:], in1=zero_sb[:], op=ALU.max)
            nc.scalar.sqrt(out=B_t[:], in_=B_t[:])
            nc.vector.tensor_mul(out=A_t[:], in0=A_t[:], in1=kinv_sb[:])  # mean
            nc.vector.tensor_sub(out=A_t[:], in0=A_t[:], in1=B_t[:])      # tau

            nc.vector.tensor_tensor(out=C_t[:], in0=A_t[:], in1=zs_t[:], op=ALU.is_le)
            e15 = [[64, S], [16, 4], [1, DEG - 1]]
            nc.vector.tensor_sub(out=sub_ap(B_t[:], 0, e15),
                                 in0=sub_ap(C_t[:], 0, e15),
                                 in1=sub_ap(C_t[:], 1, e15))
            nc.scalar.copy(out=sub_ap(B_t[:], DEG - 1, [[64, S], [16, 4], [1, 1]]),
                           in_=sub_ap(C_t[:], DEG - 1, [[64, S], [16, 4], [1, 1]]))
            nc.vector.tensor_mul(out=B_t[:], in0=B_t[:], in1=A_t[:])
            nc.vector.tensor_reduce(
                out=sub_ap(ts4_t[:], 0, [[4, S], [1, 4]]),
                in_=sub_ap(B_t[:], 0, [[64, S], [16, 4], [1, DEG]]),
                axis=mybir.AxisListType.X, op=ALU.add)
            nc.scalar.copy(
                out=sub_ap(C_t[:], 0, [[64, S], [16, 4], [1, 16]]),
                in_=sub_ap(ts4_t[:], 0, [[4, S], [1, 4], [0, 16]]))   # tau* rep
            nc.vector.tensor_sub(out=z[:], in0=z[:], in1=C_t[:])
            nc.vector.tensor_tensor(out=z[:], in0=z[:], in1=zero_sb[:], op=ALU.max)
            nc.vector.tensor_mul(out=z[:], in0=z[:], in1=z[:])
            nc.vector.tensor_mul(out=z[:], in0=z[:], in1=ws_sb[:])

            for tl in range(S):
                if prev is not None:
                    do_fma(prev, tl)

            prev = (gas, z, sc)

        for tl in range(S):
            do_fma(prev, tl)

    nc.compile()
    return nc


# ---------------------------------------------------------------- host prep

def softmax_np(x):
    e = np.exp(x - np.max(x))
    return e / e.sum()


def host_prep(cfg, h, src, w, fc_w, attn_w, head_weights, n_cores, n_total=N):
    n_own_real = n_total // n_cores
    T, S = cfg.T, cfg.S
    W = S * 64

    h_pad = np.zeros((NALL, DIN), np.float32)
    h_pad[:n_total] = h
    hq = h_pad.astype(ml_dtypes.bfloat16)

    # paired gather table: row r = [h_r | h_{r+NROW} - h_r]
    h_tab = np.zeros((NROW, ROWE), ml_dtypes.bfloat16)
    h_tab[:, 0:DIN] = hq[:NROW]
    h_tab[:, DIN:2 * DIN] = (hq[NROW:].astype(np.float32)
                             - hq[:NROW].astype(np.float32)).astype(ml_dtypes.bfloat16)

    fc_wT = np.ascontiguousarray(np.transpose(fc_w, (0, 2, 1))).astype(ml_dtypes.bfloat16)

    # dense per-node logit projections (f32, host): P = h.U_src, Q = h.U_dst
    fc_w32 = fc_w.astype(np.float32)
    U_src = np.stack([fc_w32[hh].T @ attn_w[hh, :DOUT] for hh in range(H)], 1)
    U_dst = np.stack([fc_w32[hh].T @ attn_w[hh, DOUT:] for hh in range(H)], 1)
    P_all = h_pad @ U_src          # [NALL, H]
    Q_all = h_pad @ U_dst          # [NALL, H]

    ws = softmax_np(head_weights.astype(np.float32))
    hcol = np.arange(W) % 64                       # within-tile col = h*16+d
    h_of = hcol // DEG
    d_of = hcol % DEG
    kinv448 = np.tile((1.0 / (d_of + 1.0))[None, :], (128, 1)).astype(np.float32)
    ws448 = np.tile(ws[h_of][None, :], (128, 1)).astype(np.float32)
    smask = np.tile((d_of != 0).astype(np.float32)[None, :], (128, 1))

    src2d = src.reshape(n_total, DEG).astype(np.int64)
    w2d = w.reshape(n_total, DEG).astype(np.float32)

    i1024 = np.arange(1024)
    dh_ = i1024 // 128
    ph_ = i1024 % 128

    q64 = np.arange(64)
    p128 = np.arange(128)
    rh = np.zeros((64, H * 128), np.float32)
    for hh in range(H):
        rh[:, hh * 128:(hh + 1) * 128] = (
            (q64[:, None] // 16 == hh) & (q64[:, None] % 16 == p128[None, :] % 16))
    rhrep = rh.astype(ml_dtypes.bfloat16)
    blk = np.zeros((128, H * 128), np.float32)
    for hh in range(H):
        blk[:, hh * 128:(hh + 1) * 128] = (p128[None, :] % 8 == p128[:, None] // 16)
    blkmask = blk.astype(ml_dtypes.bfloat16)

    in_maps = []
    for c in range(n_cores):
        lo = c * n_own_real
        hi = lo + n_own_real
        own_src = np.zeros((cfg.n_own, DEG), np.int64)
        own_src[:n_own_real] = src2d[lo:hi]
        own_w = np.zeros((cfg.n_own, DEG), np.float32)
        own_w[:n_own_real] = 0.5 * w2d[lo:hi]

        sel = (own_src >= NROW)
        row = np.where(sel, own_src - NROW, own_src)

        # edge-major slots: out partition p = (dst%8)*16 + d, row v = dst//8
        idxP = np.zeros((128, T * 128), np.int16)
        for t in range(T):
            for gh in range(2):
                dstl = (8 * gh + dh_) * 8 + ph_ // 16
                vals = row[t * 128 + dstl, ph_ % 16].astype(np.int16)
                pat = np.zeros((16, 64), np.int16)
                pat[i1024 % 16, i1024 // 16] = vals
                idxP[:, t * 128 + gh * 64:t * 128 + (gh + 1) * 64] = \
                    np.tile(pat, (8, 1))

        sf = sel.astype(np.float32)                       # [n_own, DEG]
        jj = np.arange(128) // 16
        dd16 = np.arange(128) % 16
        sfr = sf.reshape(T, 16, 8, DEG)
        selE = sfr[:, :, jj, dd16].transpose(2, 0, 1)     # [128, T, 16] edge-major
        selm = np.ascontiguousarray(
            np.repeat(selE.reshape(128, T * DEG)[:, :, None], 8, axis=2)
            .reshape(128, T * DEG * 8)).astype(ml_dtypes.bfloat16)

        # pq448: [p, t*64 + h*16 + d] = 0.5*(P[src] + Q[dst])
        own_nodes = np.zeros(cfg.n_own, np.int64)
        own_nodes[:n_own_real] = np.arange(lo, hi)
        pq = 0.5 * (P_all[own_src] + Q_all[own_nodes][:, None, :])  # [n_own, DEG, H]
        pq = pq.transpose(0, 2, 1).reshape(T, 128, 64).transpose(1, 0, 2)
        pq448 = np.ascontiguousarray(pq.reshape(128, T * 64)).astype(np.float32)

        # wq448: [p, t*64 + h*16 + d] = 0.5*w[node(t,p), d]
        w3 = own_w.reshape(T, 128, DEG).transpose(1, 0, 2)   # [128, T, DEG]
        wq448 = np.tile(w3[:, :, None, :], (1, 1, H, 1)).reshape(128, T * 64)
        wq448 = np.ascontiguousarray(wq448).astype(np.float32)

        in_maps.append({
            "h_tab": h_tab, "fc_wT": fc_wT, "pq448": pq448,
            "idxP": idxP, "selm": selm, "rhrep": rhrep, "blkmask": blkmask,
            "wq448": wq448, "kinv448": kinv448, "ws448": ws448,
            "scanmask": smask,
        })
    return in_maps


# ---------------------------------------------------------------- entry point

_PROG_CACHE = {}


def kernel(h, src, w, fc_w, attn_w, head_weights):
    h = np.asarray(h, np.float32)
    src = np.asarray(src)
    w = np.asarray(w, np.float32)
    fc_w = np.asarray(fc_w, np.float32)
    attn_w = np.asarray(attn_w, np.float32)
    head_weights = np.asarray(head_weights, np.float32)

    cfg = full_cfg()
    key = ("full",)
    if key not in _PROG_CACHE:
        _PROG_CACHE[key] = build_program(cfg, num_devices=CORES)
    nc = _PROG_CACHE[key]

    in_maps = host_prep(cfg, h, src, w, fc_w, attn_w, head_weights, CORES)

    from concourse.bass_utils import run_bass_kernel_spmd
    res = run_bass_kernel_spmd(nc, in_maps, core_ids=list(range(CORES)))

    n_own_real = N // CORES
    out = np.concatenate(
        [res.results[c]["out"][:n_own_real] for c in range(CORES)], axis=0)
    return out.astype(np.float32)

